# revision 64
# baseline (speedup 1.0000x reference)
"""Trainium2 Bass kernel for nn_Attention_layer (GNN message passing attention).

Math (see harness reference):
  x_Q = [input_x, pe_Q]  (N, 1024);  x_K = [input_x, pe_K]
  Q = x_Q @ WQ[h] + qb;  K = x_K @ WK[h] + kb;  V = input_x @ WV[h] + vb
  attn = softmax(Q K^T / 16, axis=k);  out = concat_h(attn @ V) @ lin_w.T + lin_b

Distribution: 8 NeuronCores, query-dim (N) sharded 512 rows/core; K/V work
replicated (no collectives).  Everything is computed transposed per core.

Key structure (v2):
  - softmax exp is SPLIT between ScalarE (ACT exp) and VectorE via a custom
    DVE op EXP8_ANT: scores are pre-scaled on the host (folded into WQ) so
    stored u = s/(16*8*gamma); exp(s/16) = (((u+A)u+B)u+D)^8 -- a monic cubic
    (gamma^3/6=1) + three squarings = exactly 8 DVE ALU stages.  ScalarE
    handles the first 2 heads of each 4-head group, VectorE the other 2.
  - mg-outer sweeps: heads 0-3 over all 32 k-chunks, then heads 4-7.  PV and
    Z accumulate in PSUM across the whole sweep (start/stop matmul chains),
    eliminating the per-group VectorE accumulation of v1.
  - K bias is dropped entirely (it cancels in softmax); V bias is folded into
    the final linear bias on the host (lin_w @ vb + lin_b).
  - K-projection drains run on ScalarE, V-projection drains (paired, 2 chunks
    per PSUM bank) split between engines, to balance ScalarE/VectorE load.
  - PSUM budget (8 banks): stA 2 + stB 2 + pvt 1 + zt 1 + proj pool 2.
"""

import os
import sys
import numpy as np
import ml_dtypes

for _p in ("/opt/trn_rl_repo", "/root/.axon_site/_ro/trn_rl_repo"):
    if os.path.isdir(_p) and _p not in sys.path:
        sys.path.insert(0, _p)

N = 4096
IND = 256          # input_x dim
QKD = 1024         # concat dim for Q/K projections
H = 8              # heads
HD = 32            # head dim
HID = 256          # H * HD
NCORES = 8
NQ = N // NCORES   # 512 query rows per core
SCALE = 1.0 / 16.0  # 1/sqrt(HID)

# --- EXP8 cubic constants (minimax fit of full cubic to e^v on |v|<=0.794,
# v = s*SCALE/8; monic form u = v/gamma) ---
EXP8_GAMMA = 1.8398762420069352
EXP8_A = 1.775468172224715
EXP8_B = 1.8482161545681495
EXP8_D = 0.9984578190881139
# stored score u = (s*SCALE) / (8*gamma); ScalarE needs Exp((8*gamma)*u)
Q_PRESCALE = SCALE / (8.0 * EXP8_GAMMA)
ACT_EXP_SCALE = 8.0 * EXP8_GAMMA

_CACHE = {}


def _register_exp8():
    """Register the EXP8_ANT custom DVE op (idempotent)."""
    from concourse.dve_spec import Spec, Src0, C0, C1, C2, lower, sq
    from concourse.dve_uop import DveOpSpec
    from concourse import dve_ops as dom

    if "EXP8_ANT" in dom._SUB_OPCODE_FOR_NAME:
        return next(op for op in dom.OPS if op.name == "EXP8_ANT")

    _h = ((Src0 + C0) * Src0 + C1) * Src0 + C2
    spec = Spec(
        body=sq(sq(sq(_h))),
        reference=lambda in0, in1, s0, s1, imm2: (
            (((in0.astype(np.float32) + s0) * in0 + s1) * in0 + imm2) ** 8
        ),
    )
    row = dom._CUSTOM_DVE_ROW_BASE + len(dom.OPS)
    shas = {}
    for ver in ("v3", "v4"):
        shas[ver] = DveOpSpec(
            name="EXP8_ANT", opcode=row, uops=lower(spec, ver=ver), rd1_en=False
        ).sha(ver)
    op = dom.DveOp("EXP8_ANT", spec, subdim=False, uops_sha=shas)
    dom.OPS.append(op)
    dom._SUB_OPCODE_FOR_NAME["EXP8_ANT"] = row
    dom.CUSTOM_DVE_SPECS["EXP8_ANT"] = spec
    return op


def _build_nc():
    from contextlib import ExitStack
    import concourse.bacc as bacc
    import concourse.tile as tile
    import concourse.mybir as mybir
    from concourse.bass import ds, ts

    EXP8 = _register_exp8()

    f32 = mybir.dt.float32
    bf16 = mybir.dt.bfloat16
    Exp = mybir.ActivationFunctionType.Exp
    mult = mybir.AluOpType.mult

    nc = bacc.Bacc("TRN2", target_bir_lowering=False, debug=False,
                   num_devices=NCORES)

    # ---- DRAM I/O (per-core shards prepared on host) ----
    xkT = nc.dram_tensor("xkT", [QKD, N], bf16, kind="ExternalInput")   # [x;peK]^T
    xqT = nc.dram_tensor("xqT", [QKD, NQ], bf16, kind="ExternalInput")  # [x;peQ]^T blk
    wq = nc.dram_tensor("wq", [QKD, HID], bf16, kind="ExternalInput")   # pre-scaled
    wk = nc.dram_tensor("wk", [QKD, HID], bf16, kind="ExternalInput")
    wv = nc.dram_tensor("wv", [IND, HID], bf16, kind="ExternalInput")
    lwT = nc.dram_tensor("lwT", [HID, HID], bf16, kind="ExternalInput")  # lin_w.T
    bias4 = nc.dram_tensor("bias4", [128, 8], f32, kind="ExternalInput")  # [p, 4m+i]
    out = nc.dram_tensor("out", [HID, NQ], f32, kind="ExternalOutput")   # out^T
    DBG = os.environ.get("KDEBUG", "0") == "1"
    if DBG:
        dzsb = nc.dram_tensor("dzsb", [128, 2 * NQ], f32, kind="ExternalOutput")
        dattn = nc.dram_tensor("dattn", [128, 2 * NQ], bf16, kind="ExternalOutput")
        dkt = nc.dram_tensor("dkt", [128, 2 * N], bf16, kind="ExternalOutput")
        dqt = nc.dram_tensor("dqt", [128, 2 * NQ], bf16, kind="ExternalOutput")
        dvt = nc.dram_tensor("dvt", [128, 32 * HID], bf16, kind="ExternalOutput")
        dzr = nc.dram_tensor("dzr", [36, NQ], f32, kind="ExternalOutput")
        dpsbs = nc.dram_tensor("dpsbs", [128, 2 * NQ], f32, kind="ExternalOutput")

    # Z-row gather: zt rows 32j hold Z_{4mg+j}; same rows for both sweeps
    # (sweep 0 is drained to zsb before sweep 1's start=True overwrites).
    selz_np = np.zeros((128, 4), dtype=np.float32)
    for j in range(4):
        selz_np[32 * j, j] = 1.0
    selz_dram = nc.inline_tensor(np.ascontiguousarray(selz_np), name="selz_const")
    # 1/Z broadcast: psb[32j+hd, q] = zr[j, q] (same pattern both sweeps)
    bsel_np = np.zeros((4, 128), dtype=np.float32)
    for j in range(4):
        bsel_np[j, 32 * j:32 * j + 32] = 1.0
    bsel_dram = nc.inline_tensor(bsel_np, name="bsel_const")
    ones_np = np.ones((128, 1), dtype=ml_dtypes.bfloat16)
    ones_dram = nc.inline_tensor(ones_np, name="ones_const")

    with tile.TileContext(nc) as tc, ExitStack() as ctx:
        consts = ctx.enter_context(tc.tile_pool(name="consts", bufs=1))
        big = ctx.enter_context(tc.tile_pool(name="big", bufs=1))
        ptp = ctx.enter_context(tc.tile_pool(name="ptp", bufs=4))
        stp = ctx.enter_context(tc.tile_pool(name="stp", bufs=1, space="PSUM"))

        # ---- SBUF tiles ----
        xkt = big.tile([128, 8, N], bf16, tag="xkt")       # x_K^T  (8 c-chunks)
        xqt = big.tile([128, 8, NQ], bf16, tag="xqt")      # x_Q^T block
        wqt = consts.tile([128, 8, HID], bf16, tag="wqt")
        wkt = consts.tile([128, 8, HID], bf16, tag="wkt")
        wvt = consts.tile([128, 2, HID], bf16, tag="wvt")
        lwt = consts.tile([128, 2, HID], bf16, tag="lwt")
        bt = consts.tile([128, 8], f32, tag="bt")          # [p, 4m+i]
        selz = consts.tile([128, 4], f32, tag="selz")
        bsel = consts.tile([4, 128], f32, tag="bsel")
        ones = consts.tile([128, 1], bf16, tag="ones")

        kt = big.tile([128, 2, N], bf16, tag="kt")         # K^T rows (h,hd)
        qt = big.tile([128, 2, NQ], bf16, tag="qt")        # Q^T
        vt = big.tile([128, 32, HID], bf16, tag="vt")      # V node-major
        attn = big.tile([128, 2, NQ], bf16, tag="attn")    # normalized attn_x^T
        zsb = big.tile([128, 2, NQ], f32, tag="zsb")       # zt PSUM drains
        pvsb = big.tile([128, 2, NQ], f32, tag="pvsb")     # pvt PSUM drains
        zr = big.tile([4, 2, NQ], f32, tag="zr")           # 1/Z per head
        outsb = big.tile([128, 2, NQ], f32, tag="outsb")

        # ---- persistent PSUM tiles ----
        stA = stp.tile([128, 2 * NQ], f32, tag="stA", name="stA")  # heads 4mg+0,1
        stB = stp.tile([128, 2 * NQ], f32, tag="stB", name="stB")  # heads 4mg+2,3
        pvt = stp.tile([128, NQ], f32, tag="pvt", name="pvt")      # PV accum
        zt = stp.tile([128, NQ], f32, tag="zt", name="zt")         # Z accum

        # ---- const / weight DMAs, ordered by first consumer; the sync
        # queue carries the critical path, the scalar HWDGE queue the
        # non-urgent constants ----
        xkT_r = xkT.rearrange("(c p) (n q) -> n p c q", p=128, q=512)
        xqT_r = xqT.rearrange("(c p) q -> p c q", p=128)
        wq_r = wq.rearrange("(c p) o -> p c o", p=128)
        wk_r = wk.rearrange("(c p) o -> p c o", p=128)
        nc.sync.dma_start(wqt[:, :, ds(0, 128)], wq_r[:, :, ds(0, 128)])
        nc.sync.dma_start(xqt[:, :4], xqT_r[:, :4])
        nc.sync.dma_start(xqt[:, 4:], xqT_r[:, 4:])
        nc.sync.dma_start(wkt[:, :, ds(0, 128)], wk_r[:, :, ds(0, 128)])
        nc.sync.dma_start(xkt[:, :, ds(0, 128)], xkT_r[0][:, :, ds(0, 128)])
        nc.sync.dma_start(bt[:], bias4[:])
        nc.sync.dma_start(wvt[:], wv.rearrange("(c p) o -> p c o", p=128))
        nc.sync.dma_start(xkt[:, :, ds(128, 384)], xkT_r[0][:, :, ds(128, 384)])
        nc.sync.dma_start(xkt[:, :, ts(1, 512)], xkT_r[1])
        nc.sync.dma_start(wqt[:, :, ds(128, 128)], wq_r[:, :, ds(128, 128)])
        nc.sync.dma_start(wkt[:, :, ds(128, 128)], wk_r[:, :, ds(128, 128)])
        nc.sync.dma_start(xkt[:, :, ts(2, 512)], xkT_r[2])
        nc.sync.dma_start(lwt[:], lwT.rearrange("(c p) o -> p c o", p=128))
        nc.sync.dma_start(selz[:], selz_dram[:])
        nc.sync.dma_start(bsel[:], bsel_dram[:])
        nc.sync.dma_start(ones[:], ones_dram[:])
        for n in range(3, 8):
            nc.sync.dma_start(xkt[:, :, ts(n, 512)], xkT_r[n])

        # zt rows outside {32j} are never written and must stay 0
        nc.vector.memset(zt[:], 0.0)
        # preload the ACT exp table set while DMAs land
        actwarm = consts.tile([8, 16], f32, tag="actwarm")
        nc.vector.memset(actwarm[:], 0.0)
        nc.scalar.activation(actwarm[:], actwarm[:], Exp)


        # ---- projection units (PE work + a drain on ScalarE or VectorE) ----
        qproj_open = {}

        def q_proj_half(m, half):
            if half == 0:
                ps = stp.tile([128, NQ], f32, tag="pz", bufs=2, name=f"qp{m}")
                qproj_open[m] = ps
            else:
                ps = qproj_open.pop(m)
            for c in range(4 * half, 4 * half + 4):
                nc.tensor.matmul(ps[:, :NQ], wqt[:, c, ts(m, 128)], xqt[:, c, :],
                                 start=(c == 0), stop=(c == 7))
            if half == 1:
                nc.vector.tensor_scalar_add(qt[:, m, :], ps[:, :NQ],
                                            bt[:, 4 * m + 0:4 * m + 1])

        def q_proj_unit(m):
            q_proj_half(m, 0)
            q_proj_half(m, 1)

        def k_proj_narrow(m, lo, w):
            ps = stp.tile([128, NQ], f32, tag="pz", bufs=2, name=f"kn{m}_{lo}")
            for c in range(8):
                nc.tensor.matmul(ps[:, :w], wkt[:, c, ts(m, 128)],
                                 xkt[:, c, ds(lo, w)],
                                 start=(c == 0), stop=(c == 7))
            nc.scalar.copy(kt[:, m, ds(lo, w)], ps[:, :w])

        kproj_open = {}

        def k_proj_quarter(n, m, qtr):
            if qtr == 0:
                ps = stp.tile([128, NQ], f32, tag="pz", bufs=2, name=f"kp{n}_{m}")
                kproj_open[(n, m)] = ps
            else:
                ps = kproj_open[(n, m)]
            for c in range(2 * qtr, 2 * qtr + 2):
                nc.tensor.matmul(ps[:, :512], wkt[:, c, ts(m, 128)],
                                 xkt[:, c, ts(n, 512)],
                                 start=(c == 0), stop=(c == 7))

        def k_proj_drain(n, m, half):
            # half-column drains fit inside ScalarE's per-group idle slack
            ps = kproj_open[(n, m)]
            nc.scalar.copy(kt[:, m, ds(512 * n + 256 * half, 256)],
                           ps[:, ds(256 * half, 256)])
            if half == 1:
                del kproj_open[(n, m)]

        vpair_open = {}

        def v_pair_mm(t):
            # projects node-chunks 2t and 2t+1 into one PSUM bank
            ps = stp.tile([128, NQ], f32, tag="pz", bufs=2, name=f"vp{t}")
            vpair_open[t] = ps
            for i in range(2):
                kcc = 2 * t + i
                for c in range(2):
                    nc.tensor.matmul(ps[:, ds(256 * i, 256)],
                                     xkt[:, c, ds(128 * kcc, 128)],
                                     wvt[:, c, :], start=(c == 0), stop=(c == 1))
            nc.vector.tensor_copy(out=vt[:, 2 * t, :], in_=ps[:, ds(0, 256)])

        def v_pair_drain2(t):
            ps = vpair_open.pop(t)
            nc.vector.tensor_copy(out=vt[:, 2 * t + 1, :], in_=ps[:, ds(256, 256)])

        def v_pair(t):
            v_pair_mm(t)
            v_pair_drain2(t)

        # ---- minimal prologue: what group (kc=0, mg=0) needs ----
        q_proj_half(0, 0)
        q_proj_half(0, 1)
        k_proj_narrow(0, 0, 128)
        v_pair(0)
        k_proj_narrow(0, 128, 384)

        # ---- scheduled PE side-work, keyed by group index g = 32*mg + kc ----
        pre_work = {}

        def at(g, fn):
            pre_work.setdefault(g, []).append(fn)

        # K-proj m=0 for node tiles 1..7: quarters at groups 4(n-1)..4(n-1)+3,
        # drain halves at the qtr3 group and the one after
        for n in range(1, 8):
            for qtr in range(4):
                at(4 * (n - 1) + qtr,
                   lambda n=n, qtr=qtr: k_proj_quarter(n, 0, qtr))
            at(4 * (n - 1) + 3, lambda n=n: k_proj_drain(n, 0, 0))
            at(4 * (n - 1) + 4, lambda n=n: k_proj_drain(n, 0, 1))
        # K-proj m=1 for node tiles 0..7: quarters at 28+4n..28+4n+3
        for n in range(8):
            for qtr in range(4):
                at(28 + 4 * n + qtr,
                   lambda n=n, qtr=qtr: k_proj_quarter(n, 1, qtr))
            at(28 + 4 * n + 3, lambda n=n: k_proj_drain(n, 1, 0))
            at(28 + 4 * n + 4, lambda n=n: k_proj_drain(n, 1, 1))
        # V pairs at odd groups (kproj owns one pz bank at all times;
        # vpairs cycle through the other), second drain next group
        for t in range(1, 16):
            at(2 * t - 1, lambda t=t: v_pair_mm(t))
            at(2 * t, lambda t=t: v_pair_drain2(t))
        # Q proj for heads 4-7
        at(21, lambda: q_proj_unit(1))

        # per-sweep epilogue, split into pieces so no engine FIFO stalls:
        # drains -> Z gather -> reciprocal -> 1/Z broadcast -> normalize
        ep_open = {}

        def ep_drain(mg):
            nc.vector.tensor_copy(out=zsb[:, mg, :], in_=zt[:])
            nc.scalar.copy(pvsb[:, mg, :], pvt[:])

        def ep_zq(mg):
            zq = stp.tile([128, NQ], f32, tag="pz", bufs=2, name=f"zq{mg}")
            ep_open[("zq", mg)] = zq
            nc.tensor.matmul(zq[ds(0, 4), :NQ], selz[:],
                             zsb[:, mg, :], start=True, stop=True)

        def ep_recip(mg):
            zq = ep_open.pop(("zq", mg))
            nc.vector.reciprocal_approx_fast(zr[:, mg, :], zq[ds(0, 4), :NQ])

        def ep_psb(mg):
            psb = stp.tile([128, NQ], f32, tag="pz", bufs=2, name=f"psb{mg}")
            ep_open[("psb", mg)] = psb
            nc.tensor.matmul(psb[:, :NQ], bsel[:], zr[:, mg, :],
                             start=True, stop=True)

        def ep_norm(mg):
            psb = ep_open.pop(("psb", mg))
            nc.vector.tensor_tensor(attn[:, mg, :], pvsb[:, mg, :],
                                    psb[:, :NQ], mult)

        post_work = {32: [lambda: ep_drain(0)]}
        at(34, lambda: ep_zq(0))
        at(35, lambda: ep_recip(0))
        at(36, lambda: ep_psb(0))
        at(37, lambda: ep_norm(0))

        # ---- main loop: 2 sweeps (mg) x 32 k-chunks; per group 4 heads ----
        def pvz_half(pt, kc, mg, half):
            # heads 2*half..2*half+1 only (their pt cols come from one engine)
            first, last = (kc == 0), (kc == 31)
            for j in (2 * half, 2 * half + 1):
                h = 4 * mg + j
                nc.tensor.matmul(
                    pvt[ds(32 * j, 32), :],
                    vt[:, kc, ds(32 * h, 32)],
                    pt[:, ts(j, NQ)],
                    start=first, stop=last,
                    tile_position=(0, 32 * j))
            for j in (2 * half, 2 * half + 1):
                nc.tensor.matmul(
                    zt[ds(32 * j, 1), :],
                    ones[:],
                    pt[:, ts(j, NQ)],
                    start=first, stop=last,
                    tile_position=(0, 32 * j))

        def pvz_unit(pt, kc, mg):
            pvz_half(pt, kc, mg, 0)
            pvz_half(pt, kc, mg, 1)

        # Loop body order matters: the PE queue is in-order, so the
        # WAR-waiting scores matmuls go LAST each group — PV/Z of the
        # previous group and projection work keep the PE busy (and its
        # p-state ramped) while the exp of group g-1 drains.
        prev = None
        for g in range(64):
            mg, kc = g // 32, g % 32
            pt = ptp.tile([128, 4 * NQ], bf16, tag="pt", name="pt")
            # heads 4mg+0,4mg+1 -> stA -> ScalarE exp
            for jj in range(2):
                nc.tensor.matmul(
                    stA[:, ts(jj, NQ)],
                    kt[ds(32 * jj, 32), mg, ds(128 * kc, 128)],
                    qt[ds(32 * jj, 32), mg, :],
                    start=True, stop=True,
                    tile_position=(32 * jj, 0))
            nc.scalar.activation(pt[:, ds(0, 2 * NQ)], stA[:], Exp,
                                 scale=ACT_EXP_SCALE)
            # heads 4mg+2,4mg+3 -> stB -> VectorE exp8
            for jj in range(2):
                j = 2 + jj
                nc.tensor.matmul(
                    stB[:, ts(jj, NQ)],
                    kt[ds(32 * j, 32), mg, ds(128 * kc, 128)],
                    qt[ds(32 * j, 32), mg, :],
                    start=True, stop=True,
                    tile_position=(32 * j, 0))
            if os.environ.get("NO_DVE_EXP", "0") == "1":
                nc.scalar.activation(pt[:, ds(2 * NQ, 2 * NQ)], stB[:], Exp,
                                     scale=ACT_EXP_SCALE)
            else:
                nc.vector._custom_dve(EXP8, out=pt[:, ds(2 * NQ, 2 * NQ)],
                                      in0=stB[:], s0=EXP8_A, s1=EXP8_B,
                                      imm2=EXP8_D)
            for fn in pre_work.get(g, []):
                fn()
            if prev is not None:
                pvz_unit(*prev)
            for fn in post_work.get(g, []):
                fn()
            prev = (pt, kc, mg)
        # last group: Z matmuls first so the tail's Z chain starts earlier
        lpt, lkc, lmg = prev
        for j in range(4):
            nc.tensor.matmul(zt[ds(32 * j, 1), :], ones[:], lpt[:, ts(j, NQ)],
                             start=False, stop=True, tile_position=(0, 32 * j))
        for j in range(4):
            h = 4 * lmg + j
            nc.tensor.matmul(pvt[ds(32 * j, 32), :], vt[:, lkc, ds(32 * h, 32)],
                             lpt[:, ts(j, NQ)], start=False, stop=True,
                             tile_position=(0, 32 * j))

        # ---- epilogue: mg1 chain pipelined at query-half granularity ----
        HQ = NQ // 2
        nc.vector.tensor_copy(out=zsb[:, 1, ds(0, HQ)], in_=zt[:, ds(0, HQ)])
        nc.scalar.copy(pvsb[:, 1, ds(0, HQ)], pvt[:, ds(0, HQ)])
        nc.vector.tensor_copy(out=zsb[:, 1, ds(HQ, HQ)], in_=zt[:, ds(HQ, HQ)])
        nc.scalar.copy(pvsb[:, 1, ds(HQ, HQ)], pvt[:, ds(HQ, HQ)])
        zq1 = stp.tile([128, NQ], f32, tag="pz", bufs=2, name="zq1")
        psb1 = stp.tile([128, NQ], f32, tag="pz", bufs=2, name="psb1")
        for h in range(2):
            sl = ds(h * HQ, HQ)
            nc.tensor.matmul(zq1[ds(0, 4), sl], selz[:], zsb[:, 1, sl],
                             start=True, stop=True)
            nc.vector.reciprocal_approx_fast(zr[:, 1, sl], zq1[ds(0, 4), sl])
            nc.tensor.matmul(psb1[:, sl], bsel[:], zr[:, 1, sl],
                             start=True, stop=True)
            nc.vector.tensor_tensor(attn[:, 1, sl], pvsb[:, 1, sl],
                                    psb1[:, sl], mult)
        if DBG:
            nc.sync.dma_start(dattn.rearrange("p (m q) -> p m q", m=2), attn[:])
            nc.sync.dma_start(dzsb.rearrange("p (m q) -> p m q", m=2), zsb[:])
            nc.sync.dma_start(dkt.rearrange("p (m q) -> p m q", m=2), kt[:])
            nc.sync.dma_start(dqt.rearrange("p (m q) -> p m q", m=2), qt[:])
            nc.sync.dma_start(dvt.rearrange("p (m q) -> p m q", m=32), vt[:])
            nc.sync.dma_start(dzr[ds(0, 4), :], zr[:, 0, :])
            nc.sync.dma_start(dpsbs.rearrange("p (m q) -> p m q", m=2), pvsb[:])
        out_r = out.rearrange("(m p) q -> p m q", p=128)
        lin0 = stp.tile([128, NQ], f32, tag="pz", bufs=2, name="lin0")
        lin1 = stp.tile([128, NQ], f32, tag="pz", bufs=2, name="lin1")
        for h in range(2):
            sl = ds(h * HQ, HQ)
            for mo, ps in ((0, lin0), (1, lin1)):
                for c in range(2):
                    nc.tensor.matmul(ps[:, sl], lwt[:, c, ts(mo, 128)],
                                     attn[:, c, sl], start=(c == 0),
                                     stop=(c == 1))
                if mo == 0:
                    nc.vector.tensor_scalar_add(outsb[:, mo, sl], ps[:, sl],
                                                bt[:, 4 * mo + 3:4 * mo + 4])
                else:
                    nc.scalar.activation(outsb[:, mo, sl], ps[:, sl],
                                         mybir.ActivationFunctionType.Identity,
                                         bias=bt[:, 4 * mo + 3:4 * mo + 4])
                if mo == 0:
                    nc.sync.dma_start(out_r[:, mo, sl], outsb[:, mo, sl])
                else:
                    nc.scalar.dma_start(out_r[:, mo, sl], outsb[:, mo, sl])

    nc.compile()
    return nc


def _get_nc():
    if "nc" not in _CACHE:
        _CACHE["nc"] = _build_nc()
    return _CACHE["nc"]


def _prep_in_maps(input_x, pe_Q, pe_K, WQ, WK, WV, Q_bias, K_bias, V_bias,
                  lin_w, lin_b):
    bf = ml_dtypes.bfloat16
    x_kT = np.ascontiguousarray(
        np.concatenate([input_x, pe_K], axis=1).T.astype(bf))       # [1024, 4096]
    x_q = np.concatenate([input_x, pe_Q], axis=1)                   # [4096, 1024]
    # WQ pre-scaled so stored scores u = s*SCALE/(8*gamma)
    wq2 = np.ascontiguousarray(
        (WQ * np.float32(Q_PRESCALE)).transpose(1, 0, 2)
        .reshape(QKD, HID).astype(bf))                              # [d,(h,hd)]
    wk2 = np.ascontiguousarray(WK.transpose(1, 0, 2).reshape(QKD, HID).astype(bf))
    wv2 = np.ascontiguousarray(WV.transpose(1, 0, 2).reshape(IND, HID).astype(bf))
    lwTn = np.ascontiguousarray(lin_w.T.astype(bf))                 # [in, out]
    # bias4 columns: 4m+0 = Q bias (pre-scaled), 4m+3 = lin_b + lin_w @ vb
    # (V bias folded into the final linear; K bias cancels in softmax).
    bias4 = np.zeros((128, 8), np.float32)
    qb = (Q_bias.reshape(HID) * np.float32(Q_PRESCALE)).astype(np.float32)
    lb = (lin_b.reshape(HID) + lin_w @ V_bias.reshape(HID)).astype(np.float32)
    for m in range(2):
        bias4[:, 4 * m + 0] = qb[128 * m:128 * (m + 1)]
        bias4[:, 4 * m + 3] = lb[128 * m:128 * (m + 1)]
    in_maps = []
    for i in range(NCORES):
        xqT_i = np.ascontiguousarray(
            x_q[i * NQ:(i + 1) * NQ].T.astype(bf))                  # [1024, 512]
        in_maps.append({
            "xkT": x_kT, "xqT": xqT_i, "wq": wq2, "wk": wk2, "wv": wv2,
            "lwT": lwTn, "bias4": bias4,
        })
    return in_maps


def _ensure_ntff_hook():
    """The agent image's antenv lacks axon_hooks; synthesize it from the
    boot script's ctypes NTFF implementation so trace=True works."""
    import types
    try:
        from antenv.axon_hooks import get_axon_ntff_profile_hook  # noqa: F401
        return
    except ImportError:
        pass
    sys.path.insert(0, "/root/.axon_site/trn_agent_boot")
    import trn_boot
    hook = trn_boot._ntff_profile_via_ctypes(
        os.environ.get("PJRT_LIBRARY_PATH", "/opt/axon/libaxon_pjrt.so"))
    mod = types.ModuleType("antenv.axon_hooks")
    mod._hook = hook
    mod.get_axon_ntff_profile_hook = lambda: mod._hook
    mod.set_axon_ntff_profile_hook = lambda h: setattr(mod, "_hook", h)
    sys.modules["antenv.axon_hooks"] = mod


def _run(in_maps, trace=False):
    from concourse.bass_utils import run_bass_kernel_spmd
    if trace:
        _ensure_ntff_hook()
    nc = _get_nc()
    res = run_bass_kernel_spmd(nc, in_maps, core_ids=list(range(NCORES)),
                               trace=trace)
    return res


def kernel(input_x, pe_Q, pe_K, A, WQ, WK, WV, Q_bias, K_bias, V_bias,
           lin_w, lin_b):
    in_maps = _prep_in_maps(
        np.asarray(input_x, np.float32), np.asarray(pe_Q, np.float32),
        np.asarray(pe_K, np.float32), np.asarray(WQ, np.float32),
        np.asarray(WK, np.float32), np.asarray(WV, np.float32),
        np.asarray(Q_bias, np.float32), np.asarray(K_bias, np.float32),
        np.asarray(V_bias, np.float32), np.asarray(lin_w, np.float32),
        np.asarray(lin_b, np.float32))
    res = _run(in_maps)
    out_full = np.empty((N, HID), np.float32)
    for i in range(NCORES):
        out_full[i * NQ:(i + 1) * NQ] = res.results[i]["out"].T
    return out_full


def hw_exec_ns(input_x, pe_Q, pe_K, A, WQ, WK, WV, Q_bias, K_bias, V_bias,
               lin_w, lin_b):
    """Run once with NTFF tracing; returns (exec_time_ns, results)."""
    in_maps = _prep_in_maps(
        np.asarray(input_x, np.float32), np.asarray(pe_Q, np.float32),
        np.asarray(pe_K, np.float32), np.asarray(WQ, np.float32),
        np.asarray(WK, np.float32), np.asarray(WV, np.float32),
        np.asarray(Q_bias, np.float32), np.asarray(K_bias, np.float32),
        np.asarray(V_bias, np.float32), np.asarray(lin_w, np.float32),
        np.asarray(lin_b, np.float32))
    res = _run(in_maps, trace=True)
    return res.exec_time_ns, res


# revision 66
# speedup vs baseline: 1.0126x; 1.0126x over previous
"""Trainium2 Bass kernel for nn_Attention_layer (GNN message passing attention).

Math (see harness reference):
  x_Q = [input_x, pe_Q]  (N, 1024);  x_K = [input_x, pe_K]
  Q = x_Q @ WQ[h] + qb;  K = x_K @ WK[h] + kb;  V = input_x @ WV[h] + vb
  attn = softmax(Q K^T / 16, axis=k);  out = concat_h(attn @ V) @ lin_w.T + lin_b

Distribution: 8 NeuronCores, query-dim (N) sharded 512 rows/core; K/V work
replicated (no collectives).  Everything is computed transposed per core.

Key structure (v2):
  - softmax exp is SPLIT between ScalarE (ACT exp) and VectorE via a custom
    DVE op EXP8_ANT: scores are pre-scaled on the host (folded into WQ) so
    stored u = s/(16*8*gamma); exp(s/16) = (((u+A)u+B)u+D)^8 -- a monic cubic
    (gamma^3/6=1) + three squarings = exactly 8 DVE ALU stages.  ScalarE
    handles the first 2 heads of each 4-head group, VectorE the other 2.
  - mg-outer sweeps: heads 0-3 over all 32 k-chunks, then heads 4-7.  PV and
    Z accumulate in PSUM across the whole sweep (start/stop matmul chains),
    eliminating the per-group VectorE accumulation of v1.
  - K bias is dropped entirely (it cancels in softmax); V bias is folded into
    the final linear bias on the host (lin_w @ vb + lin_b).
  - K-projection drains run on ScalarE, V-projection drains (paired, 2 chunks
    per PSUM bank) split between engines, to balance ScalarE/VectorE load.
  - PSUM budget (8 banks): stA 2 + stB 2 + pvt 1 + zt 1 + proj pool 2.
"""

import os
import sys
import numpy as np
import ml_dtypes

for _p in ("/opt/trn_rl_repo", "/root/.axon_site/_ro/trn_rl_repo"):
    if os.path.isdir(_p) and _p not in sys.path:
        sys.path.insert(0, _p)

N = 4096
IND = 256          # input_x dim
QKD = 1024         # concat dim for Q/K projections
H = 8              # heads
HD = 32            # head dim
HID = 256          # H * HD
NCORES = 8
NQ = N // NCORES   # 512 query rows per core
SCALE = 1.0 / 16.0  # 1/sqrt(HID)

# --- EXP8 cubic constants (minimax fit of full cubic to e^v on |v|<=0.794,
# v = s*SCALE/8; monic form u = v/gamma) ---
EXP8_GAMMA = 1.8398762420069352
EXP8_A = 1.775468172224715
EXP8_B = 1.8482161545681495
EXP8_D = 0.9984578190881139
# stored score u = (s*SCALE) / (8*gamma); ScalarE needs Exp((8*gamma)*u)
Q_PRESCALE = SCALE / (8.0 * EXP8_GAMMA)
ACT_EXP_SCALE = 8.0 * EXP8_GAMMA

_CACHE = {}


def _register_exp8():
    """Register the EXP8_ANT custom DVE op (idempotent)."""
    from concourse.dve_spec import Spec, Src0, C0, C1, C2, lower, sq
    from concourse.dve_uop import DveOpSpec
    from concourse import dve_ops as dom

    if "EXP8_ANT" in dom._SUB_OPCODE_FOR_NAME:
        return next(op for op in dom.OPS if op.name == "EXP8_ANT")

    _h = ((Src0 + C0) * Src0 + C1) * Src0 + C2
    spec = Spec(
        body=sq(sq(sq(_h))),
        reference=lambda in0, in1, s0, s1, imm2: (
            (((in0.astype(np.float32) + s0) * in0 + s1) * in0 + imm2) ** 8
        ),
    )
    row = dom._CUSTOM_DVE_ROW_BASE + len(dom.OPS)
    shas = {}
    for ver in ("v3", "v4"):
        shas[ver] = DveOpSpec(
            name="EXP8_ANT", opcode=row, uops=lower(spec, ver=ver), rd1_en=False
        ).sha(ver)
    op = dom.DveOp("EXP8_ANT", spec, subdim=False, uops_sha=shas)
    dom.OPS.append(op)
    dom._SUB_OPCODE_FOR_NAME["EXP8_ANT"] = row
    dom.CUSTOM_DVE_SPECS["EXP8_ANT"] = spec
    return op


def _build_nc():
    from contextlib import ExitStack
    import concourse.bacc as bacc
    import concourse.tile as tile
    import concourse.mybir as mybir
    from concourse.bass import ds, ts

    EXP8 = _register_exp8()

    f32 = mybir.dt.float32
    bf16 = mybir.dt.bfloat16
    Exp = mybir.ActivationFunctionType.Exp
    mult = mybir.AluOpType.mult

    nc = bacc.Bacc("TRN2", target_bir_lowering=False, debug=False,
                   num_devices=NCORES)

    # ---- DRAM I/O (per-core shards prepared on host) ----
    xkT = nc.dram_tensor("xkT", [QKD, N], bf16, kind="ExternalInput")   # [x;peK]^T
    xqT = nc.dram_tensor("xqT", [QKD, NQ], bf16, kind="ExternalInput")  # [x;peQ]^T blk
    wq = nc.dram_tensor("wq", [QKD, HID], bf16, kind="ExternalInput")   # pre-scaled
    wk = nc.dram_tensor("wk", [QKD, HID], bf16, kind="ExternalInput")
    wv = nc.dram_tensor("wv", [IND, HID], bf16, kind="ExternalInput")
    lwT = nc.dram_tensor("lwT", [HID, HID], bf16, kind="ExternalInput")  # lin_w.T
    bias4 = nc.dram_tensor("bias4", [128, 8], f32, kind="ExternalInput")  # [p, 4m+i]
    out = nc.dram_tensor("out", [HID, NQ], f32, kind="ExternalOutput")   # out^T
    DBG = os.environ.get("KDEBUG", "0") == "1"
    if DBG:
        dzsb = nc.dram_tensor("dzsb", [128, 2 * NQ], f32, kind="ExternalOutput")
        dattn = nc.dram_tensor("dattn", [128, 2 * NQ], bf16, kind="ExternalOutput")
        dkt = nc.dram_tensor("dkt", [128, 2 * N], bf16, kind="ExternalOutput")
        dqt = nc.dram_tensor("dqt", [128, 2 * NQ], bf16, kind="ExternalOutput")
        dvt = nc.dram_tensor("dvt", [128, 32 * HID], bf16, kind="ExternalOutput")
        dzr = nc.dram_tensor("dzr", [36, NQ], f32, kind="ExternalOutput")
        dpsbs = nc.dram_tensor("dpsbs", [128, 2 * NQ], f32, kind="ExternalOutput")

    # Z-row gather: zt rows 32j hold Z_{4mg+j}; same rows for both sweeps
    # (sweep 0 is drained to zsb before sweep 1's start=True overwrites).
    selz_np = np.zeros((128, 4), dtype=np.float32)
    for j in range(4):
        selz_np[32 * j, j] = 1.0
    selz_dram = nc.inline_tensor(np.ascontiguousarray(selz_np), name="selz_const")
    # 1/Z broadcast: psb[32j+hd, q] = zr[j, q] (same pattern both sweeps)
    bsel_np = np.zeros((4, 128), dtype=np.float32)
    for j in range(4):
        bsel_np[j, 32 * j:32 * j + 32] = 1.0
    bsel_dram = nc.inline_tensor(bsel_np, name="bsel_const")
    ones_np = np.ones((128, 1), dtype=ml_dtypes.bfloat16)
    ones_dram = nc.inline_tensor(ones_np, name="ones_const")

    with tile.TileContext(nc) as tc, ExitStack() as ctx:
        consts = ctx.enter_context(tc.tile_pool(name="consts", bufs=1))
        big = ctx.enter_context(tc.tile_pool(name="big", bufs=1))
        ptp = ctx.enter_context(tc.tile_pool(name="ptp", bufs=4))
        stp = ctx.enter_context(tc.tile_pool(name="stp", bufs=1, space="PSUM"))

        # ---- SBUF tiles ----
        xkt = big.tile([128, 8, N], bf16, tag="xkt")       # x_K^T  (8 c-chunks)
        xqt = big.tile([128, 8, NQ], bf16, tag="xqt")      # x_Q^T block
        wqt = consts.tile([128, 8, HID], bf16, tag="wqt")
        wkt = consts.tile([128, 8, HID], bf16, tag="wkt")
        wvt = consts.tile([128, 2, HID], bf16, tag="wvt")
        lwt = consts.tile([128, 2, HID], bf16, tag="lwt")
        bt = consts.tile([128, 8], f32, tag="bt")          # [p, 4m+i]
        selz = consts.tile([128, 4], f32, tag="selz")
        bsel = consts.tile([4, 128], f32, tag="bsel")
        ones = consts.tile([128, 1], bf16, tag="ones")

        kt = big.tile([128, 2, N], bf16, tag="kt")         # K^T rows (h,hd)
        qt = big.tile([128, 2, NQ], bf16, tag="qt")        # Q^T
        vt = big.tile([128, 32, HID], bf16, tag="vt")      # V node-major
        attn = big.tile([128, 2, NQ], bf16, tag="attn")    # normalized attn_x^T
        zsb = big.tile([128, 2, NQ], f32, tag="zsb")       # zt PSUM drains
        pvsb = big.tile([128, 2, NQ], f32, tag="pvsb")     # pvt PSUM drains
        zr = big.tile([4, 2, NQ], f32, tag="zr")           # 1/Z per head
        outsb = big.tile([128, 2, NQ], f32, tag="outsb")

        # ---- persistent PSUM tiles ----
        stA = stp.tile([128, 2 * NQ], f32, tag="stA", name="stA")  # heads 4mg+0,1
        stB = stp.tile([128, 2 * NQ], f32, tag="stB", name="stB")  # heads 4mg+2,3
        pvt = stp.tile([128, NQ], f32, tag="pvt", name="pvt")      # PV accum
        zt = stp.tile([128, NQ], f32, tag="zt", name="zt")         # Z accum

        # ---- const / weight DMAs, ordered by first consumer; the sync
        # queue carries the critical path, the scalar HWDGE queue the
        # non-urgent constants ----
        xkT_r = xkT.rearrange("(c p) (n q) -> n p c q", p=128, q=512)
        xqT_r = xqT.rearrange("(c p) q -> p c q", p=128)
        wq_r = wq.rearrange("(c p) o -> p c o", p=128)
        wk_r = wk.rearrange("(c p) o -> p c o", p=128)
        nc.sync.dma_start(wqt[:, :, ds(0, 128)], wq_r[:, :, ds(0, 128)])
        nc.sync.dma_start(xqt[:, :4], xqT_r[:, :4])
        nc.sync.dma_start(xqt[:, 4:], xqT_r[:, 4:])
        nc.sync.dma_start(wkt[:, :, ds(0, 128)], wk_r[:, :, ds(0, 128)])
        nc.sync.dma_start(xkt[:, :, ds(0, 128)], xkT_r[0][:, :, ds(0, 128)])
        nc.sync.dma_start(bt[:], bias4[:])
        nc.sync.dma_start(wvt[:], wv.rearrange("(c p) o -> p c o", p=128))
        nc.sync.dma_start(xkt[:, :, ds(128, 384)], xkT_r[0][:, :, ds(128, 384)])
        nc.sync.dma_start(xkt[:, :, ts(1, 512)], xkT_r[1])
        nc.sync.dma_start(wqt[:, :, ds(128, 128)], wq_r[:, :, ds(128, 128)])
        nc.sync.dma_start(wkt[:, :, ds(128, 128)], wk_r[:, :, ds(128, 128)])
        nc.sync.dma_start(xkt[:, :, ts(2, 512)], xkT_r[2])
        nc.sync.dma_start(lwt[:], lwT.rearrange("(c p) o -> p c o", p=128))
        nc.sync.dma_start(selz[:], selz_dram[:])
        nc.sync.dma_start(bsel[:], bsel_dram[:])
        nc.sync.dma_start(ones[:], ones_dram[:])
        for n in range(3, 8):
            nc.sync.dma_start(xkt[:, :, ts(n, 512)], xkT_r[n])

        # zt rows outside {32j} are never written and must stay 0
        nc.vector.memset(zt[:], 0.0)
        # preload the ACT exp table set while DMAs land
        actwarm = consts.tile([8, 16], f32, tag="actwarm")
        nc.vector.memset(actwarm[:], 0.0)
        nc.scalar.activation(actwarm[:], actwarm[:], Exp)


        # ---- projection units (PE work + a drain on ScalarE or VectorE) ----
        qproj_open = {}

        def q_proj_half(m, half):
            if half == 0:
                ps = stp.tile([128, NQ], f32, tag="pz", bufs=2, name=f"qp{m}")
                qproj_open[m] = ps
            else:
                ps = qproj_open.pop(m)
            for c in range(4 * half, 4 * half + 4):
                nc.tensor.matmul(ps[:, :NQ], wqt[:, c, ts(m, 128)], xqt[:, c, :],
                                 start=(c == 0), stop=(c == 7))
            if half == 1:
                nc.vector.tensor_scalar_add(qt[:, m, :], ps[:, :NQ],
                                            bt[:, 4 * m + 0:4 * m + 1])

        def q_proj_unit(m):
            q_proj_half(m, 0)
            q_proj_half(m, 1)

        def k_proj_narrow(m, lo, w):
            ps = stp.tile([128, NQ], f32, tag="pz", bufs=2, name=f"kn{m}_{lo}")
            for c in range(8):
                nc.tensor.matmul(ps[:, :w], wkt[:, c, ts(m, 128)],
                                 xkt[:, c, ds(lo, w)],
                                 start=(c == 0), stop=(c == 7))
            nc.scalar.copy(kt[:, m, ds(lo, w)], ps[:, :w])

        kproj_open = {}

        def k_proj_quarter(n, m, qtr):
            if qtr == 0:
                ps = stp.tile([128, NQ], f32, tag="pz", bufs=2, name=f"kp{n}_{m}")
                kproj_open[(n, m)] = ps
            else:
                ps = kproj_open[(n, m)]
            for c in range(2 * qtr, 2 * qtr + 2):
                nc.tensor.matmul(ps[:, :512], wkt[:, c, ts(m, 128)],
                                 xkt[:, c, ts(n, 512)],
                                 start=(c == 0), stop=(c == 7))

        def k_proj_drain(n, m, half):
            # half-column drains fit inside ScalarE's per-group idle slack
            ps = kproj_open[(n, m)]
            nc.scalar.copy(kt[:, m, ds(512 * n + 256 * half, 256)],
                           ps[:, ds(256 * half, 256)])
            if half == 1:
                del kproj_open[(n, m)]

        vpair_open = {}

        def v_pair_mm(t):
            # projects node-chunks 2t and 2t+1 into one PSUM bank
            ps = stp.tile([128, NQ], f32, tag="pz", bufs=2, name=f"vp{t}")
            vpair_open[t] = ps
            for i in range(2):
                kcc = 2 * t + i
                for c in range(2):
                    nc.tensor.matmul(ps[:, ds(256 * i, 256)],
                                     xkt[:, c, ds(128 * kcc, 128)],
                                     wvt[:, c, :], start=(c == 0), stop=(c == 1))
            nc.vector.tensor_copy(out=vt[:, 2 * t, :], in_=ps[:, ds(0, 256)])

        def v_pair_drain2(t):
            ps = vpair_open.pop(t)
            nc.vector.tensor_copy(out=vt[:, 2 * t + 1, :], in_=ps[:, ds(256, 256)])

        def v_pair(t):
            v_pair_mm(t)
            v_pair_drain2(t)

        # ---- minimal prologue: what group (kc=0, mg=0) needs ----
        q_proj_half(0, 0)
        q_proj_half(0, 1)
        k_proj_narrow(0, 0, 128)
        v_pair(0)
        k_proj_narrow(0, 128, 384)

        # ---- scheduled PE side-work, keyed by group index g = 32*mg + kc ----
        pre_work = {}

        def at(g, fn):
            pre_work.setdefault(g, []).append(fn)

        # K-proj m=0 for node tiles 1..7: quarters at groups 4(n-1)..4(n-1)+3,
        # drain halves at the qtr3 group and the one after
        for n in range(1, 8):
            for qtr in range(4):
                at(4 * (n - 1) + qtr,
                   lambda n=n, qtr=qtr: k_proj_quarter(n, 0, qtr))
            at(4 * (n - 1) + 3, lambda n=n: k_proj_drain(n, 0, 0))
            at(4 * (n - 1) + 4, lambda n=n: k_proj_drain(n, 0, 1))
        # K-proj m=1 for node tiles 0..7: quarters at 28+4n..28+4n+3
        for n in range(8):
            for qtr in range(4):
                at(28 + 4 * n + qtr,
                   lambda n=n, qtr=qtr: k_proj_quarter(n, 1, qtr))
            at(28 + 4 * n + 3, lambda n=n: k_proj_drain(n, 1, 0))
            at(28 + 4 * n + 4, lambda n=n: k_proj_drain(n, 1, 1))
        # V pairs at odd groups (kproj owns one pz bank at all times;
        # vpairs cycle through the other), second drain next group
        for t in range(1, 16):
            at(2 * t - 1, lambda t=t: v_pair_mm(t))
            at(2 * t, lambda t=t: v_pair_drain2(t))
        # Q proj for heads 4-7
        at(21, lambda: q_proj_unit(1))

        # per-sweep epilogue, split into pieces so no engine FIFO stalls:
        # drains -> Z gather -> reciprocal -> 1/Z broadcast -> normalize
        ep_open = {}

        def ep_drain(mg):
            nc.vector.tensor_copy(out=zsb[:, mg, :], in_=zt[:])
            nc.scalar.copy(pvsb[:, mg, :], pvt[:])

        def ep_zq(mg):
            zq = stp.tile([128, NQ], f32, tag="pz", bufs=2, name=f"zq{mg}")
            ep_open[("zq", mg)] = zq
            nc.tensor.matmul(zq[ds(0, 4), :NQ], selz[:],
                             zsb[:, mg, :], start=True, stop=True)

        def ep_recip(mg):
            zq = ep_open.pop(("zq", mg))
            nc.vector.reciprocal_approx_fast(zr[:, mg, :], zq[ds(0, 4), :NQ])

        def ep_psb(mg):
            psb = stp.tile([128, NQ], f32, tag="pz", bufs=2, name=f"psb{mg}")
            ep_open[("psb", mg)] = psb
            nc.tensor.matmul(psb[:, :NQ], bsel[:], zr[:, mg, :],
                             start=True, stop=True)

        def ep_norm(mg):
            psb = ep_open.pop(("psb", mg))
            nc.vector.tensor_tensor(attn[:, mg, :], pvsb[:, mg, :],
                                    psb[:, :NQ], mult)

        post_work = {32: [lambda: ep_drain(0)]}
        at(34, lambda: ep_zq(0))
        at(35, lambda: ep_recip(0))
        at(36, lambda: ep_psb(0))
        at(37, lambda: ep_norm(0))

        # ---- main loop: 2 sweeps (mg) x 32 k-chunks; per group 4 heads ----
        def pvz_half(pt, kc, mg, half):
            # heads 2*half..2*half+1 only (their pt cols come from one engine)
            first, last = (kc == 0), (kc == 31)
            for j in (2 * half, 2 * half + 1):
                h = 4 * mg + j
                nc.tensor.matmul(
                    pvt[ds(32 * j, 32), :],
                    vt[:, kc, ds(32 * h, 32)],
                    pt[:, ts(j, NQ)],
                    start=first, stop=last,
                    tile_position=(0, 32 * j))
            for j in (2 * half, 2 * half + 1):
                nc.tensor.matmul(
                    zt[ds(32 * j, 1), :],
                    ones[:],
                    pt[:, ts(j, NQ)],
                    start=first, stop=last,
                    tile_position=(0, 32 * j))

        def pvz_unit(pt, kc, mg):
            pvz_half(pt, kc, mg, 0)
            pvz_half(pt, kc, mg, 1)

        # Loop body order matters: the PE queue is in-order, so the
        # WAR-waiting scores matmuls go LAST each group — PV/Z of the
        # previous group and projection work keep the PE busy (and its
        # p-state ramped) while the exp of group g-1 drains.
        prev = None
        for g in range(64):
            mg, kc = g // 32, g % 32
            pt = ptp.tile([128, 4 * NQ], bf16, tag="pt", name="pt")
            # heads 4mg+0,4mg+1 -> stA -> ScalarE exp
            for jj in range(2):
                nc.tensor.matmul(
                    stA[:, ts(jj, NQ)],
                    kt[ds(32 * jj, 32), mg, ds(128 * kc, 128)],
                    qt[ds(32 * jj, 32), mg, :],
                    start=True, stop=True,
                    tile_position=(32 * jj, 0))
            nc.scalar.activation(pt[:, ds(0, 2 * NQ)], stA[:], Exp,
                                 scale=ACT_EXP_SCALE)
            # heads 4mg+2,4mg+3 -> stB -> VectorE exp8
            for jj in range(2):
                j = 2 + jj
                nc.tensor.matmul(
                    stB[:, ts(jj, NQ)],
                    kt[ds(32 * j, 32), mg, ds(128 * kc, 128)],
                    qt[ds(32 * j, 32), mg, :],
                    start=True, stop=True,
                    tile_position=(32 * j, 0))
            if os.environ.get("NO_DVE_EXP", "0") == "1":
                nc.scalar.activation(pt[:, ds(2 * NQ, 2 * NQ)], stB[:], Exp,
                                     scale=ACT_EXP_SCALE)
            else:
                nc.vector._custom_dve(EXP8, out=pt[:, ds(2 * NQ, 2 * NQ)],
                                      in0=stB[:], s0=EXP8_A, s1=EXP8_B,
                                      imm2=EXP8_D)
            for fn in pre_work.get(g, []):
                fn()
            if prev is not None:
                pvz_unit(*prev)
            for fn in post_work.get(g, []):
                fn()
            prev = (pt, kc, mg)
        pvz_unit(*prev)

        # ---- epilogue: mg1 chain pipelined at query-half granularity ----
        HQ = NQ // 2
        nc.vector.tensor_copy(out=zsb[:, 1, ds(0, HQ)], in_=zt[:, ds(0, HQ)])
        nc.scalar.copy(pvsb[:, 1, ds(0, HQ)], pvt[:, ds(0, HQ)])
        nc.vector.tensor_copy(out=zsb[:, 1, ds(HQ, HQ)], in_=zt[:, ds(HQ, HQ)])
        nc.scalar.copy(pvsb[:, 1, ds(HQ, HQ)], pvt[:, ds(HQ, HQ)])
        zq1 = stp.tile([128, NQ], f32, tag="pz", bufs=2, name="zq1")
        psb1 = stp.tile([128, NQ], f32, tag="pz", bufs=2, name="psb1")
        for h in range(2):
            sl = ds(h * HQ, HQ)
            nc.tensor.matmul(zq1[ds(0, 4), sl], selz[:], zsb[:, 1, sl],
                             start=True, stop=True)
            nc.vector.reciprocal_approx_fast(zr[:, 1, sl], zq1[ds(0, 4), sl])
            nc.tensor.matmul(psb1[:, sl], bsel[:], zr[:, 1, sl],
                             start=True, stop=True)
            nc.vector.tensor_tensor(attn[:, 1, sl], pvsb[:, 1, sl],
                                    psb1[:, sl], mult)
        if DBG:
            nc.sync.dma_start(dattn.rearrange("p (m q) -> p m q", m=2), attn[:])
            nc.sync.dma_start(dzsb.rearrange("p (m q) -> p m q", m=2), zsb[:])
            nc.sync.dma_start(dkt.rearrange("p (m q) -> p m q", m=2), kt[:])
            nc.sync.dma_start(dqt.rearrange("p (m q) -> p m q", m=2), qt[:])
            nc.sync.dma_start(dvt.rearrange("p (m q) -> p m q", m=32), vt[:])
            nc.sync.dma_start(dzr[ds(0, 4), :], zr[:, 0, :])
            nc.sync.dma_start(dpsbs.rearrange("p (m q) -> p m q", m=2), pvsb[:])
        out_r = out.rearrange("(m p) q -> p m q", p=128)
        lin0 = stp.tile([128, NQ], f32, tag="pz", bufs=2, name="lin0")
        lin1 = stp.tile([128, NQ], f32, tag="pz", bufs=2, name="lin1")
        for h in range(2):
            sl = ds(h * HQ, HQ)
            for mo, ps in ((0, lin0), (1, lin1)):
                for c in range(2):
                    nc.tensor.matmul(ps[:, sl], lwt[:, c, ts(mo, 128)],
                                     attn[:, c, sl], start=(c == 0),
                                     stop=(c == 1))
                if mo == 0:
                    nc.vector.tensor_scalar_add(outsb[:, mo, sl], ps[:, sl],
                                                bt[:, 4 * mo + 3:4 * mo + 4])
                else:
                    nc.scalar.activation(outsb[:, mo, sl], ps[:, sl],
                                         mybir.ActivationFunctionType.Identity,
                                         bias=bt[:, 4 * mo + 3:4 * mo + 4])
                nc.sync.dma_start(out_r[:, mo, sl], outsb[:, mo, sl])

    nc.compile()
    return nc


def _get_nc():
    if "nc" not in _CACHE:
        _CACHE["nc"] = _build_nc()
    return _CACHE["nc"]


def _prep_in_maps(input_x, pe_Q, pe_K, WQ, WK, WV, Q_bias, K_bias, V_bias,
                  lin_w, lin_b):
    bf = ml_dtypes.bfloat16
    x_kT = np.ascontiguousarray(
        np.concatenate([input_x, pe_K], axis=1).T.astype(bf))       # [1024, 4096]
    x_q = np.concatenate([input_x, pe_Q], axis=1)                   # [4096, 1024]
    # WQ pre-scaled so stored scores u = s*SCALE/(8*gamma)
    wq2 = np.ascontiguousarray(
        (WQ * np.float32(Q_PRESCALE)).transpose(1, 0, 2)
        .reshape(QKD, HID).astype(bf))                              # [d,(h,hd)]
    wk2 = np.ascontiguousarray(WK.transpose(1, 0, 2).reshape(QKD, HID).astype(bf))
    wv2 = np.ascontiguousarray(WV.transpose(1, 0, 2).reshape(IND, HID).astype(bf))
    lwTn = np.ascontiguousarray(lin_w.T.astype(bf))                 # [in, out]
    # bias4 columns: 4m+0 = Q bias (pre-scaled), 4m+3 = lin_b + lin_w @ vb
    # (V bias folded into the final linear; K bias cancels in softmax).
    bias4 = np.zeros((128, 8), np.float32)
    qb = (Q_bias.reshape(HID) * np.float32(Q_PRESCALE)).astype(np.float32)
    lb = (lin_b.reshape(HID) + lin_w @ V_bias.reshape(HID)).astype(np.float32)
    for m in range(2):
        bias4[:, 4 * m + 0] = qb[128 * m:128 * (m + 1)]
        bias4[:, 4 * m + 3] = lb[128 * m:128 * (m + 1)]
    in_maps = []
    for i in range(NCORES):
        xqT_i = np.ascontiguousarray(
            x_q[i * NQ:(i + 1) * NQ].T.astype(bf))                  # [1024, 512]
        in_maps.append({
            "xkT": x_kT, "xqT": xqT_i, "wq": wq2, "wk": wk2, "wv": wv2,
            "lwT": lwTn, "bias4": bias4,
        })
    return in_maps


def _ensure_ntff_hook():
    """The agent image's antenv lacks axon_hooks; synthesize it from the
    boot script's ctypes NTFF implementation so trace=True works."""
    import types
    try:
        from antenv.axon_hooks import get_axon_ntff_profile_hook  # noqa: F401
        return
    except ImportError:
        pass
    sys.path.insert(0, "/root/.axon_site/trn_agent_boot")
    import trn_boot
    hook = trn_boot._ntff_profile_via_ctypes(
        os.environ.get("PJRT_LIBRARY_PATH", "/opt/axon/libaxon_pjrt.so"))
    mod = types.ModuleType("antenv.axon_hooks")
    mod._hook = hook
    mod.get_axon_ntff_profile_hook = lambda: mod._hook
    mod.set_axon_ntff_profile_hook = lambda h: setattr(mod, "_hook", h)
    sys.modules["antenv.axon_hooks"] = mod


def _run(in_maps, trace=False):
    from concourse.bass_utils import run_bass_kernel_spmd
    if trace:
        _ensure_ntff_hook()
    nc = _get_nc()
    res = run_bass_kernel_spmd(nc, in_maps, core_ids=list(range(NCORES)),
                               trace=trace)
    return res


def kernel(input_x, pe_Q, pe_K, A, WQ, WK, WV, Q_bias, K_bias, V_bias,
           lin_w, lin_b):
    in_maps = _prep_in_maps(
        np.asarray(input_x, np.float32), np.asarray(pe_Q, np.float32),
        np.asarray(pe_K, np.float32), np.asarray(WQ, np.float32),
        np.asarray(WK, np.float32), np.asarray(WV, np.float32),
        np.asarray(Q_bias, np.float32), np.asarray(K_bias, np.float32),
        np.asarray(V_bias, np.float32), np.asarray(lin_w, np.float32),
        np.asarray(lin_b, np.float32))
    res = _run(in_maps)
    out_full = np.empty((N, HID), np.float32)
    for i in range(NCORES):
        out_full[i * NQ:(i + 1) * NQ] = res.results[i]["out"].T
    return out_full


def hw_exec_ns(input_x, pe_Q, pe_K, A, WQ, WK, WV, Q_bias, K_bias, V_bias,
               lin_w, lin_b):
    """Run once with NTFF tracing; returns (exec_time_ns, results)."""
    in_maps = _prep_in_maps(
        np.asarray(input_x, np.float32), np.asarray(pe_Q, np.float32),
        np.asarray(pe_K, np.float32), np.asarray(WQ, np.float32),
        np.asarray(WK, np.float32), np.asarray(WV, np.float32),
        np.asarray(Q_bias, np.float32), np.asarray(K_bias, np.float32),
        np.asarray(V_bias, np.float32), np.asarray(lin_w, np.float32),
        np.asarray(lin_b, np.float32))
    res = _run(in_maps, trace=True)
    return res.exec_time_ns, res
